# revision 63
# baseline (speedup 1.0000x reference)
"""Trainium2 Bass kernel for nn_AttentionModule (sparse_attention).

Reference computation:
  q = tanh(einsum('hde,be->hbd', Query, x))          H=8 D=256 E=1536
  k = tanh(einsum('hdf,blf->hbld', Key, bank))       B=64 L=256 F=768
  s = einsum('hbld,hbd->hbl', k, q)  masked softmax over l
  out = LeakyReLU_0.4(einsum('hbl,blf->bhf', attn, bank))

Strategy (data-parallel over batch B, 8 b's per core):
 * Mask compaction: the 0/1 mask keeps <=147 of 256 bank columns per b, so
   the host gathers unmasked columns and pads to LP=148.  Padding columns
   get a -1e4 additive score bias (exp -> 0) injected as an extra matmul.
 * The dominant k-matmul runs as error-compensated fp8 (e4m3): with
   Key*32 ~ K8 + Kr and bank*8 ~ B8 + Br, kraw = K8B8 + K8Br + KrB8
   (the fp8*fp8 residual cross term is negligible).  All three terms share
   one power-of-two scale, folded into the tanh eviction's `scale`.  Each
   product pair runs as a DoubleRow matmul (2 K-tiles per instruction).
 * Narrow dims (batch 8, heads 8) ride in the moving dimension: q, score,
   and emb matmuls cost ap_size 8 or 1 per instruction instead of 256-512.
 * Softmax skips max-subtraction (|score| < 40, safe in f32) so scores can
   stay in [l, h] layout; 1/z is broadcast to [f, h] via a ones-matmul and
   applied together with LeakyReLU on the vector engine.
 * All DMA streams are host-pre-swizzled to the exact SBUF layout
   ([128, X] row-major, contiguous >=512B lines); outputs are gathered as
   [f, (b2, fc, h)] tiles and transposed on the host.
"""

import os
import numpy as np
import ml_dtypes

import concourse.bass as bass  # noqa: F401
import concourse.mybir as mybir
import concourse.tile as tile
from concourse import bacc, bass_utils

F32 = mybir.dt.float32
F16 = mybir.dt.float16
BF16 = mybir.dt.bfloat16
FP8 = mybir.dt.float8e4
AF = mybir.ActivationFunctionType
ALU = mybir.AluOpType
DR = mybir.MatmulPerfMode.DoubleRow

H, D, E, F = 8, 256, 1536, 768
B, L = 64, 256
NCORES = 8
BPC = B // NCORES          # 8 b's per core
NBP = BPC // 2             # 4 b-pairs per core
EC, FC, DC = E // 128, F // 128, D // 128   # 12, 6, 2
# Per-bp padded unmasked-column counts: the host sorts the 64 b's by count
# and fills bp0 slots with the 16 largest, so later bps get shorter l'.
# Defaults match the fixed harness input (axon-jax PRNG).
LPS_DEFAULT = (152, 132, 128, 124)
SK, SB = 32.0, 8.0         # fp8 pre-scales for Key / bank (powers of two)

# f16 fallback for the k-matmul (accuracy reference / debugging)
K16 = os.environ.get("KERNEL_K16", "0") == "1"


def _build_program(lps=LPS_DEFAULT):
    assert all(lp % 2 == 0 for lp in lps)
    lhs_ = [lp // 2 for lp in lps]     # l-chunks: two per b
    lpps = [2 * lp for lp in lps]      # (b2, l') columns per (h, dc) group
    nsk = 1 if K16 else 2              # fp8: [K8, Kr] / [Br, B8] stream pairs
    ktdt = F16 if K16 else FP8
    kt_cols = nsk * FC * D             # per-h Key cols
    bkt_cols = [nsk * FC * w for w in lpps]   # per-bp bankT cols
    bkt_off = np.cumsum([0] + bkt_cols).tolist()
    bkn_off = np.cumsum([0] + [2 * lh for lh in lhs_]).tolist()
    sb_off = np.cumsum([0] + [4 * lh for lh in lhs_]).tolist()
    tanh_scale = 1.0 if K16 else 1.0 / (SK * SB)

    nc = bacc.Bacc("TRN2", target_bir_lowering=False, debug=False,
                   enable_asserts=False, num_devices=NCORES)
    qt = nc.dram_tensor("qt", [H, 128, EC * D], F16, kind="ExternalInput").ap()
    xt = nc.dram_tensor("xt", [128, EC * BPC], F16, kind="ExternalInput").ap()
    kt = nc.dram_tensor("kt", [H, 128, kt_cols], ktdt, kind="ExternalInput").ap()
    bkt = nc.dram_tensor("bkt", [128, bkt_off[-1]], ktdt, kind="ExternalInput").ap()
    bkn = nc.dram_tensor("bkn", [bkn_off[-1], 2 * F], BF16, kind="ExternalInput").ap()
    sbias = nc.dram_tensor("sbias", [1, sb_off[-1]], F32, kind="ExternalInput").ap()
    out = nc.dram_tensor("out", [128, NBP * 2 * FC * H], F16,
                         kind="ExternalOutput").ap()

    with tile.TileContext(nc) as tc:
        with tc.tile_pool(name="const", bufs=1) as cpool, \
             tc.tile_pool(name="weights", bufs=1) as wpool, \
             tc.tile_pool(name="bktp", bufs=1) as bpool, \
             tc.tile_pool(name="bknp", bufs=1) as npool, \
             tc.tile_pool(name="ksb", bufs=1) as kpool, \
             tc.tile_pool(name="small", bufs=4) as spool, \
             tc.tile_pool(name="psK", bufs=4, space="PSUM") as psK, \
             tc.tile_pool(name="psQ", bufs=1, space="PSUM") as psQ, \
             tc.tile_pool(name="psS", bufs=3, space="PSUM") as psS:

            # ---------------- DMA: priority order -------------------------
            xt_sb = cpool.tile([128, EC * BPC], F16)
            kt_sb = [wpool.tile([128, kt_cols], ktdt, name=f"kt{h}", tag=f"kt{h}")
                     for h in range(H)]
            qt_sb = [wpool.tile([128, EC * D], F16, name=f"qt{h}", tag=f"qt{h}")
                     for h in range(H)]
            bkt_t = [bpool.tile([128, bkt_cols[bp]], ktdt,
                                name=f"bkt{bp}", tag=f"bkt{bp}")
                     for bp in range(NBP)]
            bkn_t = [[npool.tile([lhs_[bp], 2 * F], BF16,
                                 name=f"bkn{bp}_{b2}", tag=f"bkn{bp}_{b2}")
                      for b2 in range(2)] for bp in range(NBP)]
            sb_sb = cpool.tile([1, sb_off[-1]], F32)
            # f16 output: the final values are O(1), so f16 (0.05% rel)
            # halves the tail-critical out-DMA transfer; host upcasts
            o2all = cpool.tile([128, NBP * 2 * FC * H], F16)
            onesb = cpool.tile([1, BPC], F32)
            ones_mat = cpool.tile([lhs_[0], 128], BF16)

            def dma_bkt(bp, s=None):
                o = bkt_off[bp]
                w = bkt_cols[bp]
                if s is None:
                    nc.sync.dma_start(bkt_t[bp][:], bkt[:, o:o + w])
                else:
                    h2 = w // nsk
                    nc.sync.dma_start(bkt_t[bp][:, s * h2:(s + 1) * h2],
                                      bkt[:, o + s * h2:o + (s + 1) * h2])

            # kt0 K8-half and bkt0 B8-half first so T1 matmuls start early
            hk = kt_cols // nsk
            nc.sync.dma_start(kt_sb[0][:, 0:hk], kt[0, :, 0:hk])
            if nsk == 2:
                dma_bkt(0, 1)
                nc.sync.dma_start(kt_sb[1][:], kt[1])
                dma_bkt(0, 0)
                nc.sync.dma_start(kt_sb[0][:, hk:2 * hk], kt[0, :, hk:2 * hk])
            else:
                dma_bkt(0)
                nc.sync.dma_start(kt_sb[1][:], kt[1])
            nc.sync.dma_start(xt_sb[:], xt)
            for h in range(2, H):
                nc.sync.dma_start(kt_sb[h][:], kt[h])
            nc.vector.memset(onesb[:], 1.0)
            nc.vector.memset(ones_mat[:], 1.0)
            dma_bkt(1)
            for h in range(0, 2):
                nc.sync.dma_start(qt_sb[h][:], qt[h])
            dma_bkt(2)
            for h in range(2, 5):
                nc.sync.dma_start(qt_sb[h][:], qt[h])
            dma_bkt(3)
            for h in range(5, H):
                nc.sync.dma_start(qt_sb[h][:], qt[h])
            nc.sync.dma_start(sb_sb[:], sbias)
            for bp in range(NBP):
                for b2 in range(2):
                    r = bkn_off[bp] + b2 * lhs_[bp]
                    nc.sync.dma_start(bkn_t[bp][b2][:],
                                      bkn[r:r + lhs_[bp]])

            # ---------------- k = tanh(Key @ bankT), all bps --------------
            k_sb = {}

            def k_phase(bps, warm=False, post_h=None):
                if isinstance(bps, int):
                    bps = [bps]

                def t1_mms(bp, h, ps):
                    lpp = lpps[bp]
                    vb = bkt_t[bp][:].rearrange("p (s ft c) -> p s ft c",
                                                s=nsk, ft=FC)
                    vk = kt_sb[h][:].rearrange("p (s ft d) -> p s ft d",
                                               s=nsk, ft=FC)
                    vk = kt_sb[h][:].rearrange("p (s ft d) -> p s ft d",
                                               s=nsk, ft=FC)
                    for dc in range(DC):
                        g = ps[dc][:, 0:lpp]
                        if K16:
                            for ft in range(FC):
                                nc.tensor.matmul(
                                    g, vk[:, 0, ft, dc * 128:(dc + 1) * 128],
                                    vb[:, 0, ft], start=(ft == 0),
                                    stop=(ft == FC - 1))
                        else:
                            for p in range(FC // 2):
                                nc.tensor.matmul(
                                    g,
                                    vk[:, 0, 2 * p:2 * p + 2,
                                       dc * 128:(dc + 1) * 128],
                                    vb[:, 1, 2 * p:2 * p + 2],
                                    start=(p == 0), stop=False, perf_mode=DR)

                def cross_evict(bp, h, ps):
                    lpp = lpps[bp]
                    vb = bkt_t[bp][:].rearrange("p (s ft c) -> p s ft c",
                                                s=nsk, ft=FC)
                    vk = kt_sb[h][:].rearrange("p (s ft d) -> p s ft d",
                                               s=nsk, ft=FC)
                    kt_out = kpool.tile([128, 2 * lpp], F16,
                                        name=f"k{bp}_{h}", tag=f"k{bp}_{h}")
                    for dc in range(DC):
                        if not K16:
                            g = ps[dc][:, 0:lpp]
                            # cross terms: K8.Br + Kr.B8 per f-tile
                            for ft in range(FC):
                                nc.tensor.matmul(
                                    g,
                                    vk[:, :, ft, dc * 128:(dc + 1) * 128],
                                    vb[:, :, ft],
                                    start=False, stop=(ft == FC - 1),
                                    perf_mode=DR)
                        # per-dc eviction: each dc reads its own one-bank
                        # tile, so dc0 drains while dc1 is still filling
                        nc.scalar.activation(
                            kt_out[:, dc * lpp:(dc + 1) * lpp],
                            ps[dc][:, 0:lpp],
                            AF.Tanh, scale=tanh_scale)
                    k_sb[(bp, h)] = kt_out

                start_h = 0
                if warm and not K16:
                    # bp0's first two heads: T1s lead (they need only the
                    # K8/B8 slices), crosses follow, then bp1 joins once its
                    # bank stream has landed
                    tiles = {}
                    for h in range(2):
                        tiles[h] = [psK.tile([128, 512], F32,
                                             name="psk", tag="psk")
                                    for _ in range(DC)]
                        t1_mms(bps[0], h, tiles[h])
                    for h in range(2):
                        cross_evict(bps[0], h, tiles.pop(h))
                    for bp in bps[1:]:
                        for h in range(2):
                            ps = [psK.tile([128, 512], F32,
                                           name="psk", tag="psk")
                                  for _ in range(DC)]
                            t1_mms(bp, h, ps)
                            cross_evict(bp, h, ps)
                    start_h = 2
                for h in range(start_h, H):
                    for bp in bps:
                        ps = [psK.tile([128, 512], F32,
                                       name="psk", tag="psk")
                              for _ in range(DC)]
                        t1_mms(bp, h, ps)
                        cross_evict(bp, h, ps)
                    if post_h and h in post_h:
                        post_h[h]()

            # k0 alone is DMA-starved (it needs the whole kt stream for
            # 9us of PE work); interleaving k0/k1 per head doubles the PE
            # work available per kt[h] arrival
            k_phase([0], warm=True)
            for bp in range(1, NBP - 1):
                k_phase([bp])

            # ---------------- q = tanh(Query @ x), transposed -------------
            # (issued after k(2): qt has streamed in behind the k inputs, and
            # bp0-2 score pipelines then overlap the last k-phase)
            for h in range(H):
                vq = qt_sb[h][:].rearrange("p (ec d) -> p ec d", ec=EC)
                for dc in range(DC):
                    g = psq[:, (h * DC + dc) * BPC:(h * DC + dc + 1) * BPC]
                    for ec in range(EC):
                        nc.tensor.matmul(
                            g, vq[:, ec, dc * 128:(dc + 1) * 128],
                            xt_sb[:, ec * BPC:(ec + 1) * BPC],
                            start=(ec == 0), stop=(ec == EC - 1))
            q_sb = cpool.tile([128, 128], F16)
            nc.scalar.activation(q_sb[:], psq[:, 0:128], AF.Tanh)

            # ---------------- score / softmax / emb per bp ----------------
            def score_part(bp):
                lh, lp, lpp = lhs_[bp], lps[bp], lpps[bp]
                ps = psS.tile([128, 512], F32, name="mix", tag="mix")
                # scores: out [l', (b2, lc, h)], accumulate dc + pad bias
                for b2 in range(2):
                    for lc in range(2):
                        col = (b2 * 2 + lc) * H
                        boff = sb_off[bp] + (b2 * 2 + lc) * lh
                        nc.tensor.matmul(ps[0:lh, col:col + H],
                                         sb_sb[:, boff:boff + lh],
                                         onesb[:], start=True, stop=False)
                        for h in range(H):
                            for dc in range(DC):
                                nc.tensor.matmul(
                                    ps[0:lh, col + h:col + h + 1],
                                    k_sb[(bp, h)][:, dc * lpp + b2 * lp +
                                                  lc * lh:dc * lpp + b2 * lp +
                                                  lc * lh + lh],
                                    q_sb[:, (h * DC + dc) * BPC + bp * 2 + b2:
                                         (h * DC + dc) * BPC + bp * 2 + b2 + 1],
                                    start=False,
                                    stop=(h == H - 1 and dc == DC - 1))
                exp_t = spool.tile([lhs_[0], 4 * H], BF16, name="exp", tag="exp")
                nc.scalar.activation(exp_t[0:lh, :], ps[0:lh, 0:4 * H], AF.Exp)
                return ps, exp_t

            def rest_part(bp, ps, exp_t, tail=False):
                lh = lhs_[bp]
                # z[b2, h] (cols 32:48): a ones-MATRIX lhsT makes the z
                # matmul emit the column sums broadcast across all 128
                # partitions at the same ap_size-16 cost, so no separate
                # broadcast matmul (and its engine round-trip) is needed
                ev = exp_t[0:lh, :].rearrange("p (b2 lc h) -> p b2 lc h",
                                              b2=2, lc=2)
                for lc in range(2):
                    nc.tensor.matmul(ps[:, 32:48], ones_mat[0:lh, :],
                                     ev[:, :, lc],
                                     start=(lc == 0), stop=(lc == 1))
                rz = spool.tile([128, 2 * H], F32, name="rz", tag="rz")
                nc.vector.reciprocal(rz[:], ps[:, 32:48])
                # emb[f, (b2, fc, h)] (cols 192:288)
                for b2 in range(2):
                    for fc in range(FC):
                        col = 192 + (b2 * FC + fc) * H
                        for lc in range(2):
                            nc.tensor.matmul(
                                ps[:, col:col + H],
                                bkn_t[bp][b2][:, lc * F + fc * 128:
                                              lc * F + fc * 128 + 128],
                                exp_t[0:lh, (b2 * 2 + lc) * H:
                                      (b2 * 2 + lc + 1) * H],
                                start=(lc == 0), stop=(lc == 1))
                o1 = spool.tile([128, 2 * FC * H], F32, name="o1", tag="o1")
                o2 = o2all[:, bp * 2 * FC * H:(bp + 1) * 2 * FC * H]
                if tail:
                    # last bp: LeakyReLU commutes with the positive 1/z, so
                    # Prelu the raw emb on Act (free here) in parallel with
                    # the z/recip/rzb chain; one DVE multiply finishes.
                    nc.scalar.activation(o1[:], ps[:, 192:288], AF.Prelu,
                                         alpha=0.4)
                    vb = rz[:].rearrange(
                        "p (b2 one h) -> p b2 one h", b2=2,
                        one=1).broadcast_to([128, 2, FC, H])
                    nc.vector.tensor_mul(
                        o2.rearrange("p (b2 fc h) -> p b2 fc h",
                                     b2=2, fc=FC),
                        o1[:].rearrange("p (b2 fc h) -> p b2 fc h",
                                        b2=2, fc=FC), vb)
                else:
                    nc.scalar.activation(o1[:], ps[:, 192:288], AF.Prelu,
                                         alpha=0.4)
                    vb = rz[:].rearrange(
                        "p (b2 one h) -> p b2 one h", b2=2,
                        one=1).broadcast_to([128, 2, FC, H])
                    nc.vector.tensor_mul(
                        o2.rearrange("p (b2 fc h) -> p b2 fc h",
                                     b2=2, fc=FC),
                        o1[:].rearrange("p (b2 fc h) -> p b2 fc h",
                                        b2=2, fc=FC), vb)
                if bp == NBP - 1:
                    # single out-DMA: one HWDGE gen, full-rate 1536B rows
                    nc.sync.dma_start(out, o2all[:])

            pending = None
            for bp in range(NBP - 1):
                cur = (bp, *score_part(bp))
                if pending is not None:
                    rest_part(*pending)
                pending = cur
            # rest(2) rides inside k3's head loop so its softmax/output
            # chain drains while the PE is still busy with k-matmuls
            k_phase([NBP - 1], post_h={1: (lambda p=pending: rest_part(*p))})
            rest_part(NBP - 1, *score_part(NBP - 1), tail=True)

    nc.finalize()
    return nc


def _slot_plan(mask):
    """Sort b's by unmasked count (desc); bp_j takes ranks [16j, 16j+16).
    Returns (perm, lps): perm[slot] = original b, slot = c*BPC + bp*2 + b2."""
    counts = mask.sum(axis=1)
    order = np.argsort(-counts, kind="stable")
    perm = np.empty(B, dtype=np.int64)
    for j in range(NBP):
        grp = order[16 * j:16 * (j + 1)]
        for c in range(NCORES):
            perm[c * BPC + j * 2] = grp[2 * c]
            perm[c * BPC + j * 2 + 1] = grp[2 * c + 1]
    lps = tuple(max(int(2 * ((counts[order[16 * j]] + 1) // 2)), 8)
                for j in range(NBP))
    return perm, lps


def _host_prep(x, bank, mask, Query, Key, perm, lps):
    x = np.asarray(x, dtype=np.float32)
    bank = np.asarray(bank, dtype=np.float32)
    mask = np.asarray(mask)
    Query = np.asarray(Query, dtype=np.float32)
    Key = np.asarray(Key, dtype=np.float32)
    e4 = ml_dtypes.float8_e4m3
    lhs_ = [lp // 2 for lp in lps]

    # q path: f16, host-transposed (slot-ordered x)
    xs = x[perm]
    qt = np.ascontiguousarray(Query.transpose(0, 2, 1)).reshape(
        H, EC, 128, D).transpose(0, 2, 1, 3).reshape(H, 128, EC * D)
    qt = qt.astype(np.float16)

    def swz_key(Kt):  # [H, D, F] -> [H, 128(f), FC, D]
        t = np.ascontiguousarray(Kt.transpose(0, 2, 1))
        return t.reshape(H, FC, 128, D).transpose(0, 2, 1, 3)

    if K16:
        kt = swz_key(Key).reshape(H, 128, FC * D).astype(np.float16)
    else:
        Ks = Key * SK
        K8 = Ks.astype(e4)
        Kr = (Ks - K8.astype(np.float32)).astype(e4)
        kt = np.stack([swz_key(K8.astype(np.float32)),
                       swz_key(Kr.astype(np.float32))], axis=2)
        kt = kt.reshape(H, 128, 2 * FC * D).astype(e4)

    # per-(core, bp) compacted bank streams, concatenated along columns/rows
    nsk = 1 if K16 else 2
    bdt = np.float16 if K16 else e4
    bkt_cols = sum(nsk * FC * 2 * lp for lp in lps)
    in_maps = []
    for c in range(NCORES):
        bkt_c = np.zeros((128, bkt_cols), dtype=bdt)
        bkn_rows = []
        sb_c = []
        col = 0
        for bp in range(NBP):
            lp, lh = lps[bp], lhs_[bp]
            bc = np.zeros((2, lp, F), dtype=np.float32)
            bias = np.zeros((2, lp), dtype=np.float32)
            for b2 in range(2):
                bsrc = perm[c * BPC + bp * 2 + b2]
                idx = np.nonzero(mask[bsrc])[0]
                bc[b2, :len(idx)] = bank[bsrc, idx]
                bias[b2, len(idx):] = -10000.0
            # bankT swizzle: [2, lp, F] -> [128(f), s?, FC, 2, lp]
            t = np.ascontiguousarray(bc.transpose(0, 2, 1))     # [2, F, lp]
            t = t.reshape(2, FC, 128, lp).transpose(2, 1, 0, 3)  # [128,FC,2,lp]
            if K16:
                blk = t.reshape(128, FC * 2 * lp).astype(bdt)
            else:
                ts = t * SB
                t8 = ts.astype(e4)
                tr = (ts - t8.astype(np.float32)).astype(e4)
                blk = np.stack([tr, t8.astype(e4)], axis=1).reshape(
                    128, 2 * FC * 2 * lp)
            w = nsk * FC * 2 * lp
            bkt_c[:, col:col + w] = blk
            col += w
            bkn_rows.append(bc.reshape(2, 2, lh, F).transpose(0, 2, 1, 3)
                            .reshape(2 * lh, 2 * F))
            sb_c.append(bias.reshape(4 * lh))
        in_maps.append({
            "qt": qt,
            "xt": np.ascontiguousarray(
                xs[c * BPC:(c + 1) * BPC].T.reshape(EC, 128, BPC)
                .transpose(1, 0, 2).reshape(128, EC * BPC)).astype(np.float16),
            "kt": kt,
            "bkt": bkt_c,
            "bkn": np.ascontiguousarray(np.concatenate(bkn_rows, axis=0))
            .astype(ml_dtypes.bfloat16),
            "sbias": np.concatenate(sb_c)[None, :].astype(np.float32),
        })
    return in_maps


_NC_CACHE = {}


def kernel(x, bank, mask, Query, Key):
    mask = np.asarray(mask)
    perm, lps = _slot_plan(mask)
    if lps not in _NC_CACHE:
        _NC_CACHE[lps] = _build_program(lps)
    nc = _NC_CACHE[lps]
    in_maps = _host_prep(x, bank, mask, Query, Key, perm, lps)

    trace = os.environ.get("KERNEL_TRACE", "0") == "1"
    res = bass_utils.run_bass_kernel_spmd(nc, in_maps,
                                          core_ids=list(range(NCORES)),
                                          trace=trace)
    if trace:
        print("exec_time_ns:", res.exec_time_ns,
              "mean:", res.mean_exec_time_ns,
              "core:", res.max_exec_time_core_id)
    full = np.empty((B, H, F), dtype=np.float32)
    for c, r in enumerate(res.results):
        a = r["out"].astype(np.float32).reshape(128, NBP, 2, FC, H)
        full[perm[c * BPC:(c + 1) * BPC]] = (
            a.transpose(1, 2, 4, 3, 0).reshape(BPC, H, F))
    return np.ascontiguousarray(full)
